# revision 45
# baseline (speedup 1.0000x reference)
"""BiGCN layer kernel for 8 Trainium2 NeuronCores.

Hybrid sharding, fp16 streams, both collectives fully overlapped:
  - bw direction: COLUMN-parallel SpMM. Core c owns contraction slice
    n in [c*512, (c+1)*512) of the bw adjacencies (host-pretransposed to
    [n_loc, m] fp16). Partial feats for all m accumulate per-relation in
    PSUM, stage to DRAM in fp16, and a ReduceScatter sums them while the
    fw stream runs. Starts with zero collective dependency, so PJRT launch
    skew and the kernel entry barrier hide under it.
  - fw direction: ROW-parallel SpMM. Core c owns output rows; needs the
    full fw supports, which are computed locally per n-slice, cast to fp16
    and AllGathered right after the entry barrier - completing while the
    bw stream runs. feats accumulate entirely in PSUM fp32 (96 matmuls into
    2 banks), so no trailing collective exists: the kernel tail is just
    bias+relu+final linear.
  - fp16 streams halve HBM traffic; fp16's 11-bit mantissa matches the
    fp32r precision class for these [0,1) adjacency values. All matmul
    accumulation is fp32 in PSUM; the final linear runs in fp32r; the
    residual adds an exact fp32 copy of inps^T.
  - Output is produced transposed ([h, m]) per core so the per-relation
    bias is a per-partition scalar (fused into one scalar-engine
    activation) and no on-chip transposes are needed anywhere; the host
    assembles the 8 transposed blocks.
"""

import numpy as np

N, H, R = 4096, 512, 3
K = H // 2            # 256
NC = 8                # cores
NB = N // NC          # 512 rows (m / n_loc) per core
MC = 512              # bw: m-chunk width per PSUM accumulation group
NTG = 8               # fw: n-tiles per adjacency DMA chunk

_BUILT = {}


def _build_nc():
    """Build (and cache) the Bass program. Identical program on all 8 cores."""
    if "nc" in _BUILT:
        return _BUILT["nc"]

    import concourse.bass as bass
    import concourse.mybir as mybir
    from concourse import bacc, tile

    f32 = mybir.dt.float32
    f32r = mybir.dt.float32r
    f16 = mybir.dt.float16
    nc = bacc.Bacc(None, num_devices=NC)

    inpsT = nc.dram_tensor("inpsT", [H, NB], f16, kind="ExternalInput")
    inpsR = nc.dram_tensor("inpsR", [H, NB], f32, kind="ExternalInput")
    adjTb = nc.dram_tensor("adjTb", [R, NB, N], f16, kind="ExternalInput")
    adjTf = nc.dram_tensor("adjTf", [R, N, NB], f16, kind="ExternalInput")
    wst = nc.dram_tensor("wst", [2 * R, H, K], f16, kind="ExternalInput")
    bstack = nc.dram_tensor("bstack", [4, 128, R], f32, kind="ExternalInput")
    w1 = nc.dram_tensor("w1", [H, H], f32r, kind="ExternalInput")
    b1s = nc.dram_tensor("b1s", [4, 128, 1], f32, kind="ExternalInput")
    outT = nc.dram_tensor("outT", [H, NB], f32, kind="ExternalOutput")

    HT = H // 128     # 4 h-tiles
    NT = NB // 128    # 4 local n-tiles
    JT = H // 128     # 4 output j tiles
    NMC = N // MC     # 4 bw m-chunks
    NGRP = N // (NTG * 128)   # 4 fw chunks per relation
    Relu = mybir.ActivationFunctionType.Relu
    Identity = mybir.ActivationFunctionType.Identity

    with tile.TileContext(nc) as tc:
        with (
            tc.tile_pool(name="const", bufs=1) as const,
            tc.tile_pool(name="adjp", bufs=6) as adjp,
            tc.tile_pool(name="supf", bufs=1) as supfp,
            tc.tile_pool(name="evacp", bufs=3) as evacp,
            tc.tile_pool(name="psum", bufs=2, space=bass.MemorySpace.PSUM) as psump,
            tc.tile_pool(name="psbw", bufs=4, space=bass.MemorySpace.PSUM) as psbwp,
            tc.tile_pool(name="psacc", bufs=2, space=bass.MemorySpace.PSUM) as psaccp,
            tc.tile_pool(name="dram", bufs=1, space="DRAM") as dramp,
        ):
            # ---------------- constants into SBUF ----------------
            inpsT_sb = const.tile([128, HT, NB], f16)       # [p_h, ht, n_loc]
            nc.sync.dma_start(inpsT_sb[:], inpsT[:, :].rearrange("(t p) n -> p t n", p=128))
            wst_sb = const.tile([128, 2 * R, HT, K], f16)   # [p_h, r, ht, k]
            nc.sync.dma_start(wst_sb[:], wst[:, :, :].rearrange("r (t p) k -> p r t k", p=128))
            inpsR_sb = const.tile([128, HT, NB], f32)       # exact fp32 for residual
            nc.scalar.dma_start(inpsR_sb[:], inpsR[:, :].rearrange("(t p) n -> p t n", p=128))
            w1_sb = const.tile([128, HT, H], f32r)          # [p_h, ht, j]
            nc.scalar.dma_start(w1_sb[:], w1[:, :].rearrange("(t p) j -> p t j", p=128))
            bst_sb = const.tile([128, JT, R], f32)
            nc.scalar.dma_start(bst_sb[:], bstack[:, :, :].rearrange("t p r -> p t r"))
            b1_sb = const.tile([128, JT], f32)
            nc.scalar.dma_start(b1_sb[:], b1s[:, :, :].rearrange("t p o -> p (t o)"))

            # summed (over relations) concat bias, per (p, jt)
            bias_sb = const.tile([128, JT], f32)
            for jt in range(JT):
                nc.vector.tensor_add(
                    bias_sb[:, jt : jt + 1], bst_sb[:, jt, 0:1], bst_sb[:, jt, 1:2]
                )
                nc.vector.tensor_add(
                    bias_sb[:, jt : jt + 1], bias_sb[:, jt : jt + 1], bst_sb[:, jt, 2:3]
                )

            # ---------------- local supports ----------------
            # bw supports stay on-core in SBUF (column-parallel needs only the
            # local n-slice); fw supports stage to DRAM fp16 and AllGather.
            supbw_sb = const.tile([128, R, NT, K], f16)     # [p_n, r, nt, k]
            supb_fw = dramp.tile([R, NB, K], f16, name="supb_fw", tag="supb_fw")
            agsup = dramp.tile(
                [NC, R, NB, K], f16, name="agsup", tag="agsup", addr_space="Shared"
            )
            for dirn in (1, 0):                             # fw first: feeds the AG
                for ri in range(R):
                    r = dirn * R + ri
                    for nt in range(NT):
                        ps = psump.tile([128, K], f32, tag="ps")
                        for ht in range(HT):
                            nc.tensor.matmul(
                                ps[:],
                                inpsT_sb[:, ht, nt * 128 : (nt + 1) * 128],
                                wst_sb[:, r, ht, :],
                                start=(ht == 0),
                                stop=(ht == HT - 1),
                            )
                        if dirn == 1:
                            sv = evacp.tile([128, K], f16, tag="sv")
                            nc.vector.tensor_copy(sv[:], ps[:])
                            nc.scalar.dma_start(
                                supb_fw[ri, nt * 128 : (nt + 1) * 128, :], sv[:]
                            )
                        else:
                            nc.vector.tensor_copy(supbw_sb[:, ri, nt, :], ps[:])
                if dirn == 1:
                    nc.gpsimd.collective_compute(
                        "AllGather",
                        mybir.AluOpType.bypass,
                        replica_groups=[list(range(NC))],
                        ins=[supb_fw[:].opt()],
                        outs=[agsup[:].opt()],
                    )

            # ---------------- bw: column-parallel stream + ReduceScatter ----------
            stag = dramp.tile([NC, K, NB], f16, name="stag", tag="stag")
            for mc in range(NMC):
                ps0 = psbwp.tile([128, MC], f32, tag="pb", name="ps0")  # k 0:128
                ps1 = psbwp.tile([128, MC], f32, tag="pb", name="ps1")  # k 128:256
                for ri in range(R):
                    at = adjp.tile([128, NT, MC], f16, tag="adj", name="atb")
                    nc.sync.dma_start(
                        at[:],
                        adjTb[ri, :, mc * MC : (mc + 1) * MC].rearrange(
                            "(t p) m -> p t m", p=128
                        ),
                    )
                    for nt in range(NT):
                        first = ri == 0 and nt == 0
                        last = ri == R - 1 and nt == NT - 1
                        for kk, ps in ((0, ps0), (1, ps1)):
                            nc.tensor.matmul(
                                ps[:],
                                supbw_sb[:, ri, nt, kk * 128 : (kk + 1) * 128],
                                at[:, nt, :],
                                start=first,
                                stop=last,
                            )
                for kk, ps in ((0, ps0), (1, ps1)):
                    ev = evacp.tile([128, MC], f16, tag="ev")
                    nc.vector.tensor_copy(ev[:], ps[:])
                    nc.scalar.dma_start(
                        stag[mc, kk * 128 : (kk + 1) * 128, :], ev[:]
                    )
            rs_out = dramp.tile([1, K, NB], f16, name="rs_out", tag="rs_out")
            nc.gpsimd.collective_compute(
                "ReduceScatter",
                mybir.AluOpType.add,
                replica_groups=[list(range(NC))],
                ins=[stag[:].opt()],
                outs=[rs_out[:].opt()],
            )

            # ---------------- fw: row-parallel stream, PSUM-resident ----------
            facc = [
                psaccp.tile([128, NB], f32, tag="facc", name=f"facc{kk}")
                for kk in range(2)
            ]
            # SWDGE (gpsimd) queue: won't sit behind the bw staging writes on
            # the scalar HWDGE queue, so fw matmuls unblock right after the AG
            sf = supfp.tile([128, NC, R, NT, K], f16, tag="sf", name="sf")
            for cp in range(4):
                nc.gpsimd.dma_start(
                    sf[:, 2 * cp : 2 * cp + 2, :, :, :],
                    agsup[2 * cp : 2 * cp + 2, :, :, :].rearrange(
                        "c r (t p) k -> p c r t k", p=128
                    ),
                )
            for ri in range(R):
                for g in range(NGRP):
                    at = adjp.tile([128, NTG, NB], f16, tag="adj", name="atf")
                    nc.sync.dma_start(
                        at[:],
                        adjTf[ri, g * NTG * 128 : (g + 1) * NTG * 128, :].rearrange(
                            "(g p) m -> p g m", p=128
                        ),
                    )
                    for gg in range(NTG):
                        ntg = g * NTG + gg          # global n-tile 0..31
                        c2, ntl = ntg // NT, ntg % NT
                        first = ri == 0 and ntg == 0
                        last = ri == R - 1 and ntg == N // 128 - 1
                        for kk in range(2):
                            nc.tensor.matmul(
                                facc[kk][:],
                                sf[:, c2, ri, ntl, kk * 128 : (kk + 1) * 128],
                                at[:, gg, :],
                                start=first,
                                stop=last,
                            )

            # ---------------- bias + relu + final linear + residual ----------------
            # bw half (ht 0,1) comes from rs_out, fw half (ht 2,3) from facc PSUM.
            frelu_sb = const.tile([128, HT, NB], f32r)      # [p_h, ht, m_loc]
            for jt in range(HT):
                if jt < 2:
                    ft = evacp.tile([128, NB], f16, tag="ftmp")
                    nc.scalar.dma_start(
                        ft[:], rs_out[0, jt * 128 : (jt + 1) * 128, :]
                    )
                    src = ft[:]
                else:
                    src = facc[jt - 2][:]
                nc.scalar.activation(
                    frelu_sb[:, jt, :], src, Relu, bias=bias_sb[:, jt : jt + 1]
                )
            for jt in range(JT):
                pso = psump.tile([128, NB], f32, tag="ps", name=f"pso{jt}")
                for ht in range(HT):
                    nc.tensor.matmul(
                        pso[:],
                        w1_sb[:, ht, jt * 128 : (jt + 1) * 128],
                        frelu_sb[:, ht, :],
                        start=(ht == 0),
                        stop=(ht == HT - 1),
                    )
                ot = evacp.tile([128, NB], f32, tag="ev")
                nc.scalar.activation(
                    ot[:], pso[:], Identity, bias=b1_sb[:, jt : jt + 1]
                )
                nc.vector.tensor_add(ot[:], ot[:], inpsR_sb[:, jt, :])
                nc.sync.dma_start(outT[jt * 128 : (jt + 1) * 128, :], ot[:])

    nc.compile()
    nc.finalize()
    _BUILT["nc"] = nc
    return nc


def _round_fp32r(a):
    """Round fp32 to the fp32r (TF32-like, 1s+8e+11m in top 20 bits) format
    with round-to-nearest-even, as the PE's fp32r datapath expects."""
    b = np.ascontiguousarray(a, np.float32).view(np.uint32).astype(np.uint64)
    lsb = (b >> 12) & 1
    r = ((b + 0x7FF + lsb) & 0xFFFFF000).astype(np.uint32)
    return r.view(np.float32)


def _make_in_maps(inps, fw_adjs, bw_adjs, W_fw, b_fw, W_bw, b_bw, W1, b1):
    f = np.float32
    inps = np.asarray(inps, f)
    W1 = _round_fp32r(np.asarray(W1, f))
    wst = np.ascontiguousarray(
        np.concatenate([np.asarray(W_bw, f), np.asarray(W_fw, f)], axis=0),
        np.float16,
    )
    b_cat = np.concatenate([np.asarray(b_bw, f), np.asarray(b_fw, f)], axis=1)  # [R, H]
    bstack = np.ascontiguousarray(b_cat.T.reshape(4, 128, R))
    b1s = np.ascontiguousarray(np.asarray(b1, f).reshape(4, 128, 1))
    fw_adjs = np.asarray(fw_adjs, f)
    bw_adjs = np.asarray(bw_adjs, f)

    in_maps = []
    for c in range(NC):
        sl = slice(c * NB, (c + 1) * NB)
        adjTb_c = np.empty((R, NB, N), np.float16)   # bw: [n_loc, m] column shard
        adjTf_c = np.empty((R, N, NB), np.float16)   # fw: [n, m_loc] row shard
        for r in range(R):
            adjTb_c[r] = bw_adjs[r][:, sl].T
            adjTf_c[r] = fw_adjs[r][sl, :].T
        in_maps.append(
            {
                "inpsT": np.ascontiguousarray(inps[sl].T, np.float16),
                "inpsR": np.ascontiguousarray(inps[sl].T),
                "adjTb": adjTb_c,
                "adjTf": adjTf_c,
                "wst": wst,
                "bstack": bstack,
                "w1": W1,
                "b1s": b1s,
            }
        )
    return in_maps


def run(trace=False, **inputs):
    """Run the SPMD kernel; returns (full_output, BassKernelResults)."""
    from concourse.bass_utils import run_bass_kernel_spmd

    nc = _build_nc()
    in_maps = _make_in_maps(**inputs)
    res = run_bass_kernel_spmd(nc, in_maps, core_ids=list(range(NC)), trace=trace)
    out = np.empty((N, H), np.float32)
    for c in range(NC):
        out[c * NB : (c + 1) * NB] = res.results[c]["outT"].T
    return out, res


def kernel(**inputs):
    out, _ = run(trace=False, **inputs)
    return out


# revision 47
# speedup vs baseline: 1.2200x; 1.2200x over previous
"""BiGCN layer kernel for 8 Trainium2 NeuronCores.

Strategy (1D column-parallel SpMM, fp16 streams, ReduceScatter epilogue):
  - Each core c owns the contraction slice n in [c*512, (c+1)*512) of all six
    adjacency matrices (3 bw + 3 fw), pre-transposed on host to [n_loc, m]
    fp16 so the contraction dim lands on SBUF partitions with no on-chip
    transposes. fp16 halves the dominant HBM traffic; its 11-bit mantissa
    matches the fp32r (TF32-like) precision class for these [0,1) values.
  - sup[r] = inps @ W[r] is computed locally per core for its n-slice only
    (no support gather: the column-parallel form needs just the local slice,
    so the streams start with zero collective dependency and PJRT launch
    skew + the kernel entry barrier hide under productive work).
  - feats^T partials (all m, summed over a direction's 3 relations directly
    in PSUM) stage to DRAM in fp16 and ReduceScatter across the 8 cores;
    RS(bw) overlaps the fw stream. Core c receives its own m-block.
  - bias+relu fuse into one scalar-engine activation (bias is per-partition
    because feats is produced transposed [h, m]); the final linear runs in
    fp32r, split so its bw half overlaps RS(fw); the residual adds an exact
    fp32 copy of inps^T. Host assembles the 8 transposed output blocks.
"""

import numpy as np

N, H, R = 4096, 512, 3
K = H // 2            # 256
NC = 8                # cores
NB = N // NC          # 512 rows (m / n_loc) per core
MC = 1024             # m-chunk width streamed per PSUM accumulation group

_BUILT = {}


def _build_nc():
    """Build (and cache) the Bass program. Identical program on all 8 cores."""
    if "nc" in _BUILT:
        return _BUILT["nc"]

    import concourse.bass as bass
    import concourse.mybir as mybir
    from concourse import bacc, tile

    f32 = mybir.dt.float32
    f32r = mybir.dt.float32r
    f16 = mybir.dt.float16
    nc = bacc.Bacc(None, num_devices=NC)

    inpsT = nc.dram_tensor("inpsT", [H, NB], f16, kind="ExternalInput")
    inpsR = nc.dram_tensor("inpsR", [H, NB], f32, kind="ExternalInput")
    adjT = nc.dram_tensor("adjT", [2 * R, NB, N], f16, kind="ExternalInput")
    wst = nc.dram_tensor("wst", [2 * R, H, K], f16, kind="ExternalInput")
    bstack = nc.dram_tensor("bstack", [4, 128, R], f32, kind="ExternalInput")
    w1 = nc.dram_tensor("w1", [H, H], f32r, kind="ExternalInput")
    b1s = nc.dram_tensor("b1s", [4, 128, 1], f32, kind="ExternalInput")
    outT = nc.dram_tensor("outT", [H, NB], f32, kind="ExternalOutput")

    HT = H // 128     # 4 h-tiles
    NT = NB // 128    # 4 n_loc tiles
    JT = H // 128     # 4 output j tiles
    NMC = N // MC     # 4 m chunks
    Relu = mybir.ActivationFunctionType.Relu
    Identity = mybir.ActivationFunctionType.Identity

    with tile.TileContext(nc) as tc:
        with (
            tc.tile_pool(name="const", bufs=1) as const,
            tc.tile_pool(name="adjp", bufs=6) as adjp,
            tc.tile_pool(name="evacp", bufs=3) as evacp,
            tc.tile_pool(name="psum", bufs=4, space=bass.MemorySpace.PSUM) as psump,
            tc.tile_pool(name="dram", bufs=1, space="DRAM") as dramp,
        ):
            # ---------------- constants into SBUF ----------------
            inpsT_sb = const.tile([128, HT, NB], f16)       # [p_h, ht, n_loc]
            nc.sync.dma_start(inpsT_sb[:], inpsT[:, :].rearrange("(t p) n -> p t n", p=128))
            wst_sb = const.tile([128, 2 * R, HT, K], f16)   # [p_h, r, ht, k]
            nc.sync.dma_start(wst_sb[:], wst[:, :, :].rearrange("r (t p) k -> p r t k", p=128))
            inpsR_sb = const.tile([128, HT, NB], f32)       # exact fp32 for residual
            nc.scalar.dma_start(inpsR_sb[:], inpsR[:, :].rearrange("(t p) n -> p t n", p=128))
            w1_sb = const.tile([128, HT, H], f32r)          # [p_h, ht, j]
            nc.scalar.dma_start(w1_sb[:], w1[:, :].rearrange("(t p) j -> p t j", p=128))
            bst_sb = const.tile([128, JT, R], f32)
            nc.scalar.dma_start(bst_sb[:], bstack[:, :, :].rearrange("t p r -> p t r"))
            b1_sb = const.tile([128, JT], f32)
            nc.scalar.dma_start(b1_sb[:], b1s[:, :, :].rearrange("t p o -> p (t o)"))

            # summed (over relations) concat bias, per (p, jt)
            bias_sb = const.tile([128, JT], f32)
            for jt in range(JT):
                nc.vector.tensor_add(
                    bias_sb[:, jt : jt + 1], bst_sb[:, jt, 0:1], bst_sb[:, jt, 1:2]
                )
                nc.vector.tensor_add(
                    bias_sb[:, jt : jt + 1], bias_sb[:, jt : jt + 1], bst_sb[:, jt, 2:3]
                )

            # ---------------- local supports: sup[r][n_loc, k] ----------------
            sup_sb = const.tile([128, 2 * R, NT, K], f16)   # [p_n, r, nt, k]
            for r in range(2 * R):
                for nt in range(NT):
                    ps = psump.tile([128, K], f32, tag="pb")
                    for ht in range(HT):
                        nc.tensor.matmul(
                            ps[:],
                            inpsT_sb[:, ht, nt * 128 : (nt + 1) * 128],
                            wst_sb[:, r, ht, :],
                            start=(ht == 0),
                            stop=(ht == HT - 1),
                        )
                    nc.vector.tensor_copy(sup_sb[:, r, nt, :], ps[:])

            # ---------------- adjacency stream + RS staging ----------------
            # Two separate staging tensors: one shared tile would make the fw
            # stream's staging writes serialize behind RS(bw)'s read.
            stag0 = dramp.tile([NC, K, NB], f16, name="stag0", tag="stag0")
            stag1 = dramp.tile([NC, K, NB], f16, name="stag1", tag="stag1")
            stags = [stag0, stag1]
            rs_out = []
            for dirn in range(2):                           # 0 = bw (h 0:256), 1 = fw
                stag = stags[dirn]
                for mc in range(NMC):
                    ps0 = psump.tile([128, MC], f32, tag="pb", name="ps0")  # k 0:128
                    ps1 = psump.tile([128, MC], f32, tag="pb", name="ps1")  # k 128:256
                    for ri in range(R):
                        r = dirn * R + ri
                        at = adjp.tile([128, NT, MC], f16, tag="adj")
                        nc.sync.dma_start(
                            at[:],
                            adjT[r, :, mc * MC : (mc + 1) * MC].rearrange(
                                "(t p) m -> p t m", p=128
                            ),
                        )
                        for nt in range(NT):
                            first = ri == 0 and nt == 0
                            last = ri == R - 1 and nt == NT - 1
                            for kk, ps in ((0, ps0), (1, ps1)):
                                lhsT = sup_sb[:, r, nt, kk * 128 : (kk + 1) * 128]
                                for mh in range(MC // 512):
                                    nc.tensor.matmul(
                                        ps[:, mh * 512 : (mh + 1) * 512],
                                        lhsT,
                                        at[:, nt, mh * 512 : (mh + 1) * 512],
                                        start=first,
                                        stop=last,
                                    )
                    for kk, ps in ((0, ps0), (1, ps1)):
                        ev = evacp.tile([128, MC], f16, tag="ev")
                        nc.vector.tensor_copy(ev[:], ps[:])
                        for d2 in range(MC // NB):
                            dest = (mc * MC) // NB + d2
                            nc.scalar.dma_start(
                                stag[dest, kk * 128 : (kk + 1) * 128, :],
                                ev[:, d2 * NB : (d2 + 1) * NB],
                            )
                ro = dramp.tile(
                    [1, K, NB], f16, name=f"rs_out{dirn}", tag=f"rs_out{dirn}"
                )
                rs_out.append(ro)
                nc.gpsimd.collective_compute(
                    "ReduceScatter",
                    mybir.AluOpType.add,
                    replica_groups=[list(range(NC))],
                    ins=[stag[:].opt()],
                    outs=[ro[:].opt()],
                )

            # ---------------- bias + relu + final linear + residual ----------------
            # The final matmul accumulates per h-direction so the bw half
            # (frelu ht 0,1 from RS(bw)) runs while RS(fw) is still in flight.
            frelu_sb = const.tile([128, HT, NB], f32r)      # [p_h, ht, m_loc]
            psos = []
            for half in range(2):                           # 0: ht 0,1 (bw), 1: ht 2,3
                for jt2 in range(2):
                    jt = half * 2 + jt2
                    ft = evacp.tile([128, NB], f16, tag="ftmp")
                    nc.scalar.dma_start(ft[:], rs_out[half][0, jt2 * 128 : (jt2 + 1) * 128, :])
                    nc.scalar.activation(
                        frelu_sb[:, jt, :], ft[:], Relu, bias=bias_sb[:, jt : jt + 1]
                    )
                for jt in range(JT):
                    if half == 0:
                        psos.append(
                            psump.tile([128, NB], f32, tag="pb", name=f"pso{jt}")
                        )
                    pso = psos[jt]
                    for ht in (half * 2, half * 2 + 1):
                        nc.tensor.matmul(
                            pso[:],
                            w1_sb[:, ht, jt * 128 : (jt + 1) * 128],
                            frelu_sb[:, ht, :],
                            start=(ht == 0),
                            stop=(ht == HT - 1),
                        )
            for jt in range(JT):
                ot = evacp.tile([128, NB], f32, tag="ev")
                nc.scalar.activation(
                    ot[:], psos[jt][:], Identity, bias=b1_sb[:, jt : jt + 1]
                )
                nc.vector.tensor_add(ot[:], ot[:], inpsR_sb[:, jt, :])
                nc.sync.dma_start(outT[jt * 128 : (jt + 1) * 128, :], ot[:])

    nc.compile()
    nc.finalize()
    _BUILT["nc"] = nc
    return nc


def _round_fp32r(a):
    """Round fp32 to the fp32r (TF32-like, 1s+8e+11m in top 20 bits) format
    with round-to-nearest-even, as the PE's fp32r datapath expects."""
    b = np.ascontiguousarray(a, np.float32).view(np.uint32).astype(np.uint64)
    lsb = (b >> 12) & 1
    r = ((b + 0x7FF + lsb) & 0xFFFFF000).astype(np.uint32)
    return r.view(np.float32)


def _make_in_maps(inps, fw_adjs, bw_adjs, W_fw, b_fw, W_bw, b_bw, W1, b1):
    f = np.float32
    inps = np.asarray(inps, f)
    W1 = _round_fp32r(np.asarray(W1, f))
    wst = np.ascontiguousarray(
        np.concatenate([np.asarray(W_bw, f), np.asarray(W_fw, f)], axis=0),
        np.float16,
    )
    b_cat = np.concatenate([np.asarray(b_bw, f), np.asarray(b_fw, f)], axis=1)  # [R, H]
    bstack = np.ascontiguousarray(b_cat.T.reshape(4, 128, R))
    b1s = np.ascontiguousarray(np.asarray(b1, f).reshape(4, 128, 1))
    fw_adjs = np.asarray(fw_adjs, f)
    bw_adjs = np.asarray(bw_adjs, f)

    in_maps = []
    for c in range(NC):
        sl = slice(c * NB, (c + 1) * NB)
        adjT_c = np.empty((2 * R, NB, N), np.float16)
        for r in range(R):
            adjT_c[r] = bw_adjs[r][:, sl].T
            adjT_c[R + r] = fw_adjs[r][:, sl].T
        in_maps.append(
            {
                "inpsT": np.ascontiguousarray(inps[sl].T, np.float16),
                "inpsR": np.ascontiguousarray(inps[sl].T),
                "adjT": adjT_c,
                "wst": wst,
                "bstack": bstack,
                "w1": W1,
                "b1s": b1s,
            }
        )
    return in_maps


def run(trace=False, **inputs):
    """Run the SPMD kernel; returns (full_output, BassKernelResults)."""
    from concourse.bass_utils import run_bass_kernel_spmd

    nc = _build_nc()
    in_maps = _make_in_maps(**inputs)
    res = run_bass_kernel_spmd(nc, in_maps, core_ids=list(range(NC)), trace=trace)
    out = np.empty((N, H), np.float32)
    for c in range(NC):
        out[c * NB : (c + 1) * NB] = res.results[c]["outT"].T
    return out, res


def kernel(**inputs):
    out, _ = run(trace=False, **inputs)
    return out
